# revision 14
# baseline (speedup 1.0000x reference)
"""BERT self-attention on 8 Trainium2 NeuronCores.

Sharding: data-parallel over batch (4 cores per batch element) x
tensor-parallel over heads (4 heads per core). Q/K/V projections are
column-sharded, the output projection is row-sharded; each core returns a
partial [S, D] output which the host sums (+ b_o).

Per-core math (batch b, heads hs = 4 heads, cols = 256 feature slice):
  QT = Wq_sl.T @ X_b.T        [256, 2048]   (bias added per-partition)
  KT = Wk_sl.T @ X_b.T        [256, 2048]
  V  = X_b @ Wv_sl            [2048, 256] -> V' = [V_h | 1] per head, mask
                              folded in multiplicatively
  per head h, q-block qb (512 wide):
    S^T tile [128k, 512q] = K_h @ Q_h^T slices  (PSUM)
    expS = exp(S^T / 8)                          (ACT, PSUM->SBUF)
    O'   = V'_h.T @ expS   accumulated over 16 k-tiles -> [65, 512]
           rows 0:64 = unnormalized O_h^T, row 64 = softmax denominator
  normalize: O^T *= 1/den (denominator broadcast across partitions via a
  stride-0 DMA read from a DRAM staging buffer)
  Y_partial = O^T.T @ Wo_sl   [2048, 1024]

All matmuls run as float32r (FP22 reduced precision, full PE rate,
fp32 PSUM accumulation).
"""

import sys

for _p in ("/root/.axon_site/_ro/trn_rl_repo", "/opt/trn_rl_repo"):
    if _p not in sys.path:
        sys.path.append(_p)

import numpy as np

B, S, D, H, DH = 2, 2048, 1024, 16, 64
P = 128
NCORES = 8
HPC = 4              # heads per core
CW = HPC * DH        # 256: per-core feature slice width
DK = D // P          # 8 k-tiles over the model dim
SP = S // P          # 16 s-tiles
NB = 4               # 512-wide blocks over S
NW = S // NB         # 512
G = 3                # exp kt-group size (PSUM banks per stage tile)

_STATE = {}


def _build_nc():
    import concourse.bacc as bacc
    import concourse.tile as tile
    from concourse import mybir

    f32 = mybir.dt.float32
    f32r = mybir.dt.float32r
    bf16 = mybir.dt.bfloat16
    Exp = mybir.ActivationFunctionType.Exp

    nc = bacc.Bacc(None, target_bir_lowering=False, debug=False)

    with tile.TileContext(nc) as tc:
        with tc.tile_pool(name="dram", bufs=1, space="DRAM") as dram:
            xt = dram.tile([D, S], f32, kind="ExternalInput", name="xt", uniquify=False)
            wq = dram.tile([P, DK, CW], f32, kind="ExternalInput", name="wq", uniquify=False)
            wk = dram.tile([P, DK, CW], f32, kind="ExternalInput", name="wk", uniquify=False)
            wv = dram.tile([P, DK, CW], f32, kind="ExternalInput", name="wv", uniquify=False)
            wo = dram.tile([P, CW // P, D], f32, kind="ExternalInput", name="wo", uniquify=False)
            bq = dram.tile([P, CW // P], f32, kind="ExternalInput", name="bq", uniquify=False)
            bk = dram.tile([P, CW // P], f32, kind="ExternalInput", name="bk", uniquify=False)
            bv = dram.tile([1, CW], f32, kind="ExternalInput", name="bv", uniquify=False)
            vmask = dram.tile([P, SP, HPC], f32, kind="ExternalInput", name="vmask", uniquify=False)
            y = dram.tile([S, D], f32, kind="ExternalOutput", name="y", uniquify=False)
            dden = dram.tile([HPC * NB, NW], f32, name="dden")

            import concourse.bass as bass

            consts_cm = tc.tile_pool(name="consts", bufs=1)
            consts = consts_cm.__enter__()
            xt_sb = consts.tile([P, DK, S], f32r, name="xt_sb")
            wq_sb = consts.tile([P, DK, CW], f32r, name="wq_sb")
            wk_sb = consts.tile([P, DK, CW], f32r, name="wk_sb")
            wv_sb = consts.tile([P, DK, CW], f32r, name="wv_sb")
            wo_sb = consts.tile([P, CW // P, D], f32r, name="wo_sb")
            bq_sb = consts.tile([P, CW // P], f32, name="bq_sb")
            bk_sb = consts.tile([P, CW // P], f32, name="bk_sb")
            vbias_bc = consts.tile([P, CW], f32, name="vbias_bc")
            vmask_sb = consts.tile([P, SP, HPC], f32, name="vmask_sb")
            qt_sb = consts.tile([P, CW // P, S], bf16, name="qt_sb")
            kz_sb = consts.tile([P, HPC, SP, P], bf16, name="kz_sb")
            zsrc = consts.tile([P, NW], f32, name="zsrc")
            vp_sb = consts.tile([P, SP, HPC, DH + 1], f32r, name="vp_sb")
            ot_sb = consts.tile([P, CW // P, S], f32r, name="ot_sb")

            # ---- input DMAs (small tensors first so the first matmuls
            # aren't queued behind the 8MB of X^T) ----
            nc.sync.dma_start(out=wq_sb[:], in_=wq[:].bitcast(f32r))
            nc.sync.dma_start(out=bq_sb[:], in_=bq[:])
            for k in range(DK):
                nc.sync.dma_start(out=xt_sb[:, k, :], in_=xt[k * P:(k + 1) * P, :].bitcast(f32r))
            nc.sync.dma_start(out=wk_sb[:], in_=wk[:].bitcast(f32r))
            nc.sync.dma_start(out=wv_sb[:], in_=wv[:].bitcast(f32r))
            nc.sync.dma_start(out=wo_sb[:], in_=wo[:].bitcast(f32r))
            nc.sync.dma_start(out=bk_sb[:], in_=bk[:])
            nc.sync.dma_start(out=vmask_sb[:], in_=vmask[:])
            # broadcast b_v row across all 128 partitions (stride-0 DMA read)
            bv_row = bv[0:1, :]
            bv_bcast = bass.AP(
                tensor=bv_row.tensor,
                offset=bv_row.offset,
                ap=[[0, P]] + list(bv_row.ap[1:]),
            )
            nc.sync.dma_start(out=vbias_bc[:], in_=bv_bcast)

            # zero-fill kz (stride-0 free-dim broadcast of a zeroed tile);
            # each head's K^T occupies its natural 64 partition rows, the
            # other 64 rows stay zero so the scores matmul contracts over
            # the full 128 partitions (HAM keeps the PE clock warm only
            # for full-row matmuls)
            nc.vector.memset(zsrc[:], 0.0)
            zview = zsrc[:]
            zbc = bass.AP(
                tensor=zview.tensor, offset=zview.offset,
                ap=[list(zview.ap[0]), [0, HPC * SP * P // NW]] + list(zview.ap[1:]))
            nc.vector.tensor_copy(out=kz_sb[:], in_=zbc)

            # ---- projections ----
            # QT first, k-outer (8 live PSUM accumulators) so matmuls pace
            # with the X^T DMA stream instead of waiting for all of it
            with tc.tile_pool(name="proj_psum", bufs=8, space="PSUM") as proj_psum:
                psqs = {}
                for m in range(CW // P):
                    for nb in range(NB):
                        psqs[(m, nb)] = proj_psum.tile([P, NW], f32, name="psq", tag="pp")
                for k in range(DK):
                    for m in range(CW // P):
                        for nb in range(NB):
                            nc.tensor.matmul(
                                psqs[(m, nb)][:], wq_sb[:, k, m * P:(m + 1) * P],
                                xt_sb[:, k, nb * NW:(nb + 1) * NW],
                                start=(k == 0), stop=(k == DK - 1))
                for m in range(CW // P):
                    for nb in range(NB):
                        nc.vector.tensor_scalar_add(
                            out=qt_sb[:, m, nb * NW:(nb + 1) * NW],
                            in0=psqs[(m, nb)][:], scalar1=bq_sb[:, m:m + 1])

            with tc.tile_pool(name="kproj_psum", bufs=4, space="PSUM") as kproj_psum, \
                 tc.tile_pool(name="vproj_psum", bufs=4, space="PSUM") as vproj_psum:
                for m in range(CW // P):
                    for nb in range(NB):
                        psk = kproj_psum.tile([P, NW], f32, name="psk")
                        for k in range(DK):
                            nc.tensor.matmul(
                                psk[:], wk_sb[:, k, m * P:(m + 1) * P],
                                xt_sb[:, k, nb * NW:(nb + 1) * NW],
                                start=(k == 0), stop=(k == DK - 1))
                        nc.vector.tensor_scalar_add(
                            out=kz_sb[0:DH, 2 * m, nb * 4:(nb + 1) * 4, :],
                            in0=psk[0:DH, :].rearrange("p (a b) -> p a b", a=4),
                            scalar1=bk_sb[0:DH, m:m + 1])
                        nc.vector.tensor_scalar_add(
                            out=kz_sb[DH:P, 2 * m + 1, nb * 4:(nb + 1) * 4, :],
                            in0=psk[DH:P, :].rearrange("p (a b) -> p a b", a=4),
                            scalar1=bk_sb[DH:P, m:m + 1])

                for st in range(SP):            # V s-tiles
                    psv = vproj_psum.tile([P, CW], f32, name="psv")
                    for k in range(DK):
                        nc.tensor.matmul(
                            psv[:], xt_sb[:, k, st * P:(st + 1) * P],
                            wv_sb[:, k, :],
                            start=(k == 0), stop=(k == DK - 1))
                    # bias add + scatter into [h, 65]-strided V' slots
                    nc.vector.tensor_add(
                        out=vp_sb[:, st, :, 0:DH],
                        in0=psv[:].rearrange("p (h d) -> p h d", h=HPC),
                        in1=vbias_bc[:].rearrange("p (h d) -> p h d", h=HPC))
                    # ones column times mask == mask itself
                    nc.vector.tensor_copy(
                        out=vp_sb[:, st, :, DH:DH + 1], in_=vmask_sb[:, st, :])
                    for h in range(HPC):
                        nc.vector.tensor_scalar_mul(
                            out=vp_sb[:, st, h, 0:DH], in0=vp_sb[:, st, h, 0:DH],
                            scalar1=vmask_sb[:, st, h:h + 1])

            # ---- attention + inline normalize/output projection ----
            # qb-outer so each q-block's normalize + Y projection can be
            # emitted one head-iteration behind, overlapping the next
            # block's attention and keeping the PE dense (and HAM-warm)
            kt_groups = [list(range(g * G, min(g * G + G, SP))) for g in range((SP + G - 1) // G)]

            sb_pools_cm = [
                tc.tile_pool(name="exps_pool", bufs=3),
                tc.tile_pool(name="st65_pool", bufs=2),
                tc.tile_pool(name="bcast_pool", bufs=2),
                tc.tile_pool(name="y_pool", bufs=2),
            ]
            exps_pool, st65_pool, bcast_pool, y_pool = [c.__enter__() for c in sb_pools_cm]
            with tc.tile_pool(name="stage_psum", bufs=2, space="PSUM") as stage_psum, \
                 tc.tile_pool(name="op_psum", bufs=1, space="PSUM") as op_psum, \
                 tc.tile_pool(name="y_psum", bufs=1, space="PSUM") as y_psum:

                def emit_pv(p):
                    op_p, h_p, kts_p, ex_p = p
                    for i, kt in enumerate(kts_p):
                        nc.tensor.matmul(
                            op_p[:], vp_sb[:, kt, h_p, :], ex_p[:, i, :],
                            start=(kt == 0), stop=(kt == SP - 1),
                            skip_group_check=True)

                def emit_evict(p):
                    op_p, h_p, qb_p = p
                    qs = slice(qb_p * NW, (qb_p + 1) * NW)
                    mt, po = h_p // 2, (h_p % 2) * DH
                    st65 = st65_pool.tile([DH + 1, NW], f32, name="st65")
                    nc.vector.tensor_copy(out=st65[:], in_=op_p[:])
                    nc.sync.dma_start(
                        out=ot_sb[po:po + DH, mt, qs], in_=st65[0:DH, :].bitcast(f32r))
                    nc.sync.dma_start(
                        out=dden[h_p * NB + qb_p, :], in_=st65[DH:DH + 1, :])

                def emit_normalize(qb_p, mt):
                    qs = slice(qb_p * NW, (qb_p + 1) * NW)
                    bc = bcast_pool.tile([P, NW], f32, name="bc")
                    for half in range(2):
                        hh = 2 * mt + half
                        den_row = dden[hh * NB + qb_p:hh * NB + qb_p + 1, :]
                        den_bcast = bass.AP(
                            tensor=den_row.tensor,
                            offset=den_row.offset,
                            ap=[[0, DH]] + list(den_row.ap[1:]),
                        )
                        nc.sync.dma_start(
                            out=bc[half * DH:(half + 1) * DH, :], in_=den_bcast)
                    nc.vector.reciprocal(out=bc[:], in_=bc[:])
                    nc.vector.tensor_mul(
                        out=ot_sb[:, mt, qs],
                        in0=ot_sb[:, mt, qs], in1=bc[:])

                def emit_y_st(st):
                    yt = y_pool.tile([P, D], f32, name="yt")
                    for n2 in range(2):
                        yps = y_psum.tile([P, NW], f32, name="yps")
                        for k2 in range(CW // P):
                            nc.tensor.matmul(
                                yps[:], ot_sb[:, k2, st * P:(st + 1) * P],
                                wo_sb[:, k2, n2 * NW:(n2 + 1) * NW],
                                start=(k2 == 0), stop=(k2 == CW // P - 1))
                        nc.vector.tensor_copy(
                            out=yt[:, n2 * NW:(n2 + 1) * NW], in_=yps[:])
                    nc.sync.dma_start(out=y[st * P:(st + 1) * P, :], in_=yt[:])

                from collections import deque
                pends = deque()     # (op_ps, h, kts, ex, last, qb)
                def drain_one():
                    op_p, h_p, kts_p, ex_p, last, qb_p = pends.popleft()
                    emit_pv((op_p, h_p, kts_p, ex_p))
                    if last:
                        emit_evict((op_p, h_p, qb_p))

                for qb in range(NB):
                    qs = slice(qb * NW, (qb + 1) * NW)
                    for h in range(HPC):
                        mt = h // 2
                        if h == 1 and qb > 0:
                            emit_normalize(qb - 1, 1)
                        if h == 3:
                            emit_normalize(qb, 0)
                        op_ps = op_psum.tile([DH + 1, NW], f32, name="op_ps")
                        for gi, kts in enumerate(kt_groups):
                            ng = len(kts)
                            st_ps = stage_psum.tile([P, G, NW], f32, name="st_ps")
                            for i, kt in enumerate(kts):
                                nc.tensor.matmul(
                                    st_ps[:, i, :],
                                    kz_sb[:, h, kt, :],
                                    qt_sb[:, mt, qs],
                                    start=True, stop=True)
                            if len(pends) >= 2:
                                drain_one()
                            if qb > 0 and h >= 2 and gi in (1, 3):
                                sts = 4 * (qb - 1) + 2 * (h - 2) + (gi - 1) // 2
                                emit_y_st(sts)
                            ex = exps_pool.tile([P, G, NW], f32r, name="ex")
                            nc.scalar.activation(
                                out=ex[:, 0:ng, :], in_=st_ps[:, 0:ng, :],
                                func=Exp, scale=1.0 / np.sqrt(DH))
                            pends.append((op_ps, h, kts, ex, gi == len(kt_groups) - 1, qb))
                while pends:
                    drain_one()

            # tail: last q-block's remaining normalize + output projection,
            # with the attention PSUM banks freed for deeper Y buffering
            with tc.tile_pool(name="y2_psum", bufs=4, space="PSUM") as y2_psum:
                emit_normalize(NB - 1, 1)
                for st in range(4 * (NB - 1), 4 * NB):
                    yt = y_pool.tile([P, D], f32, name="yt")
                    for n2 in range(2):
                        yps = y2_psum.tile([P, NW], f32, name="yps")
                        for k2 in range(CW // P):
                            nc.tensor.matmul(
                                yps[:], ot_sb[:, k2, st * P:(st + 1) * P],
                                wo_sb[:, k2, n2 * NW:(n2 + 1) * NW],
                                start=(k2 == 0), stop=(k2 == CW // P - 1))
                        nc.vector.tensor_copy(
                            out=yt[:, n2 * NW:(n2 + 1) * NW], in_=yps[:])
                    nc.sync.dma_start(out=y[st * P:(st + 1) * P, :], in_=yt[:])

            for c in reversed(sb_pools_cm):
                c.__exit__(None, None, None)
            consts_cm.__exit__(None, None, None)

    nc.compile()
    return nc


def _get_nc():
    if "nc" not in _STATE:
        _STATE["nc"] = _build_nc()
    return _STATE["nc"]


def _make_in_maps(hidden_states, attention_mask, W_q, b_q, W_k, b_k, W_v, b_v, W_o):
    hs = np.asarray(hidden_states, dtype=np.float32)
    mask = np.asarray(attention_mask)
    W_q = np.asarray(W_q, dtype=np.float32)
    W_k = np.asarray(W_k, dtype=np.float32)
    W_v = np.asarray(W_v, dtype=np.float32)
    W_o = np.asarray(W_o, dtype=np.float32)
    b_q = np.asarray(b_q, dtype=np.float32)
    b_k = np.asarray(b_k, dtype=np.float32)
    b_v = np.asarray(b_v, dtype=np.float32)

    in_maps = []
    for c in range(NCORES):
        b, j = c // (NCORES // B), c % (NCORES // B)
        cols = slice(CW * j, CW * (j + 1))
        xt = np.ascontiguousarray(hs[b].T)                                   # [D, S]
        wq = np.ascontiguousarray(W_q[:, cols].reshape(DK, P, CW).transpose(1, 0, 2))
        wk = np.ascontiguousarray(W_k[:, cols].reshape(DK, P, CW).transpose(1, 0, 2))
        wv = np.ascontiguousarray(W_v[:, cols].reshape(DK, P, CW).transpose(1, 0, 2))
        wo = np.ascontiguousarray(W_o[cols, :].reshape(CW // P, P, D).transpose(1, 0, 2))
        bqc = np.ascontiguousarray(b_q[cols].reshape(CW // P, P).T)          # [128, 2]
        bkc = np.ascontiguousarray(b_k[cols].reshape(CW // P, P).T)
        bvc = np.ascontiguousarray(b_v[cols].reshape(1, CW))
        m = mask[b * H + HPC * j: b * H + HPC * (j + 1), 0, :].astype(np.float32)  # [4, S]
        vm = np.ascontiguousarray(m.reshape(HPC, SP, P).transpose(2, 1, 0))  # [128, 16, 4]
        in_maps.append({
            "xt": xt, "wq": wq, "wk": wk, "wv": wv, "wo": wo,
            "bq": bqc, "bk": bkc, "bv": bvc, "vmask": vm,
        })
    return in_maps


def run(inputs, trace=False, **trace_kwargs):
    """Run the SPMD kernel. Returns (full_output, BassKernelResults)."""
    from concourse.bass_utils import run_bass_kernel_spmd

    nc = _get_nc()
    in_maps = _make_in_maps(
        inputs["hidden_states"], inputs["attention_mask"],
        inputs["W_q"], inputs["b_q"], inputs["W_k"], inputs["b_k"],
        inputs["W_v"], inputs["b_v"], inputs["W_o"])
    res = run_bass_kernel_spmd(
        nc, in_maps, list(range(NCORES)), trace=trace, **trace_kwargs)

    b_o = np.asarray(inputs["b_o"], dtype=np.float32)
    out = np.zeros((B, S, D), dtype=np.float32)
    gpb = NCORES // B
    for c in range(NCORES):
        out[c // gpb] += res.results[c]["y"]
    out += b_o[None, None, :]
    return out, res


def kernel(**inputs):
    out, _ = run(inputs, trace=False)
    return out


# revision 15
# speedup vs baseline: 1.1409x; 1.1409x over previous
"""BERT self-attention on 8 Trainium2 NeuronCores.

Sharding: data-parallel over batch (4 cores per batch element) x
tensor-parallel over heads (4 heads per core). Q/K/V projections are
column-sharded, the output projection is row-sharded; each core returns a
partial [S, D] output which the host sums (+ b_o).

Per-core math (batch b, heads hs = 4 heads, cols = 256 feature slice):
  QT = Wq_sl.T @ X_b.T        [256, 2048]   (bias added per-partition)
  KT = Wk_sl.T @ X_b.T        [256, 2048]
  V  = X_b @ Wv_sl            [2048, 256] -> V' = [V_h | 1] per head, mask
                              folded in multiplicatively
  per head h, q-block qb (512 wide):
    S^T tile [128k, 512q] = K_h @ Q_h^T slices  (PSUM)
    expS = exp(S^T / 8)                          (ACT, PSUM->SBUF)
    O'   = V'_h.T @ expS   accumulated over 16 k-tiles -> [65, 512]
           rows 0:64 = unnormalized O_h^T, row 64 = softmax denominator
  normalize: O^T *= 1/den (denominator broadcast across partitions via a
  stride-0 DMA read from a DRAM staging buffer)
  Y_partial = O^T.T @ Wo_sl   [2048, 1024]

All matmuls run as float32r (FP22 reduced precision, full PE rate,
fp32 PSUM accumulation).
"""

import sys

for _p in ("/root/.axon_site/_ro/trn_rl_repo", "/opt/trn_rl_repo"):
    if _p not in sys.path:
        sys.path.append(_p)

import numpy as np

B, S, D, H, DH = 2, 2048, 1024, 16, 64
P = 128
NCORES = 8
HPC = 4              # heads per core
CW = HPC * DH        # 256: per-core feature slice width
DK = D // P          # 8 k-tiles over the model dim
SP = S // P          # 16 s-tiles
NB = 4               # 512-wide blocks over S
NW = S // NB         # 512
G = 3                # exp kt-group size (PSUM banks per stage tile)

_STATE = {}


def _build_nc():
    import concourse.bacc as bacc
    import concourse.tile as tile
    from concourse import mybir

    f32 = mybir.dt.float32
    f32r = mybir.dt.float32r
    bf16 = mybir.dt.bfloat16
    Exp = mybir.ActivationFunctionType.Exp

    nc = bacc.Bacc(None, target_bir_lowering=False, debug=False)

    with tile.TileContext(nc) as tc:
        with tc.tile_pool(name="dram", bufs=1, space="DRAM") as dram:
            xt = dram.tile([D, S], f32, kind="ExternalInput", name="xt", uniquify=False)
            wq = dram.tile([P, DK, CW], f32, kind="ExternalInput", name="wq", uniquify=False)
            wk = dram.tile([P, DK, CW], f32, kind="ExternalInput", name="wk", uniquify=False)
            wv = dram.tile([P, DK, CW], f32, kind="ExternalInput", name="wv", uniquify=False)
            wo = dram.tile([P, CW // P, D], f32, kind="ExternalInput", name="wo", uniquify=False)
            bq = dram.tile([P, CW // P], f32, kind="ExternalInput", name="bq", uniquify=False)
            bk = dram.tile([P, CW // P], f32, kind="ExternalInput", name="bk", uniquify=False)
            bv = dram.tile([1, CW], f32, kind="ExternalInput", name="bv", uniquify=False)
            vmask = dram.tile([P, SP, HPC], f32, kind="ExternalInput", name="vmask", uniquify=False)
            y = dram.tile([S, D], f32, kind="ExternalOutput", name="y", uniquify=False)
            dden = dram.tile([HPC * NB, NW], f32, name="dden")

            import concourse.bass as bass

            consts_cm = tc.tile_pool(name="consts", bufs=1)
            consts = consts_cm.__enter__()
            xt_sb = consts.tile([P, DK, S], f32r, name="xt_sb")
            wq_sb = consts.tile([P, DK, CW], f32r, name="wq_sb")
            wk_sb = consts.tile([P, DK, CW], f32r, name="wk_sb")
            wv_sb = consts.tile([P, DK, CW], f32r, name="wv_sb")
            wo_sb = consts.tile([P, CW // P, D], f32r, name="wo_sb")
            bq_sb = consts.tile([P, CW // P], f32, name="bq_sb")
            bk_sb = consts.tile([P, CW // P], f32, name="bk_sb")
            vbias_bc = consts.tile([P, CW], f32, name="vbias_bc")
            vmask_sb = consts.tile([P, SP, HPC], f32, name="vmask_sb")
            qt_sb = consts.tile([P, CW // P, S], bf16, name="qt_sb")
            kz_sb = consts.tile([P, HPC, SP, P], bf16, name="kz_sb")
            zsrc = consts.tile([P, NW], f32, name="zsrc")
            vp_sb = consts.tile([P, SP, HPC, DH + 1], f32r, name="vp_sb")
            ot_sb = consts.tile([P, CW // P, S], f32r, name="ot_sb")

            # ---- input DMAs (small tensors first so the first matmuls
            # aren't queued behind the 8MB of X^T) ----
            nc.sync.dma_start(out=wq_sb[:], in_=wq[:].bitcast(f32r))
            nc.sync.dma_start(out=bq_sb[:], in_=bq[:])
            for k in range(DK):
                nc.sync.dma_start(out=xt_sb[:, k, :], in_=xt[k * P:(k + 1) * P, :].bitcast(f32r))
            nc.sync.dma_start(out=wk_sb[:], in_=wk[:].bitcast(f32r))
            nc.sync.dma_start(out=wv_sb[:], in_=wv[:].bitcast(f32r))
            nc.sync.dma_start(out=wo_sb[:], in_=wo[:].bitcast(f32r))
            nc.sync.dma_start(out=bk_sb[:], in_=bk[:])
            nc.sync.dma_start(out=vmask_sb[:], in_=vmask[:])
            # broadcast b_v row across all 128 partitions (stride-0 DMA read)
            bv_row = bv[0:1, :]
            bv_bcast = bass.AP(
                tensor=bv_row.tensor,
                offset=bv_row.offset,
                ap=[[0, P]] + list(bv_row.ap[1:]),
            )
            nc.sync.dma_start(out=vbias_bc[:], in_=bv_bcast)

            # zero-fill kz (stride-0 free-dim broadcast of a zeroed tile);
            # each head's K^T occupies its natural 64 partition rows, the
            # other 64 rows stay zero so the scores matmul contracts over
            # the full 128 partitions (HAM keeps the PE clock warm only
            # for full-row matmuls)
            nc.vector.memset(zsrc[:], 0.0)
            zview = zsrc[:]
            zbc = bass.AP(
                tensor=zview.tensor, offset=zview.offset,
                ap=[list(zview.ap[0]), [0, HPC * SP * P // NW]] + list(zview.ap[1:]))
            nc.vector.tensor_copy(out=kz_sb[:], in_=zbc)

            # ---- projections ----
            # QT first, k-outer (8 live PSUM accumulators) so matmuls pace
            # with the X^T DMA stream instead of waiting for all of it
            with tc.tile_pool(name="proj_psum", bufs=8, space="PSUM") as proj_psum:
                psqs = {}
                for m in range(CW // P):
                    for nb in range(NB):
                        psqs[(m, nb)] = proj_psum.tile([P, NW], f32, name="psq", tag="pp")
                for k in range(DK):
                    for m in range(CW // P):
                        for nb in range(NB):
                            nc.tensor.matmul(
                                psqs[(m, nb)][:], wq_sb[:, k, m * P:(m + 1) * P],
                                xt_sb[:, k, nb * NW:(nb + 1) * NW],
                                start=(k == 0), stop=(k == DK - 1))
                for m in range(CW // P):
                    for nb in range(NB):
                        nc.vector.tensor_scalar_add(
                            out=qt_sb[:, m, nb * NW:(nb + 1) * NW],
                            in0=psqs[(m, nb)][:], scalar1=bq_sb[:, m:m + 1])

            with tc.tile_pool(name="kproj_psum", bufs=4, space="PSUM") as kproj_psum, \
                 tc.tile_pool(name="vproj_psum", bufs=4, space="PSUM") as vproj_psum:
                for m in range(CW // P):
                    for nb in range(NB):
                        psk = kproj_psum.tile([P, NW], f32, name="psk")
                        for k in range(DK):
                            nc.tensor.matmul(
                                psk[:], wk_sb[:, k, m * P:(m + 1) * P],
                                xt_sb[:, k, nb * NW:(nb + 1) * NW],
                                start=(k == 0), stop=(k == DK - 1))
                        nc.vector.tensor_scalar_add(
                            out=kz_sb[0:DH, 2 * m, nb * 4:(nb + 1) * 4, :],
                            in0=psk[0:DH, :].rearrange("p (a b) -> p a b", a=4),
                            scalar1=bk_sb[0:DH, m:m + 1])
                        nc.vector.tensor_scalar_add(
                            out=kz_sb[DH:P, 2 * m + 1, nb * 4:(nb + 1) * 4, :],
                            in0=psk[DH:P, :].rearrange("p (a b) -> p a b", a=4),
                            scalar1=bk_sb[DH:P, m:m + 1])

                for st in range(SP):            # V s-tiles
                    psv = vproj_psum.tile([P, CW], f32, name="psv")
                    for k in range(DK):
                        nc.tensor.matmul(
                            psv[:], xt_sb[:, k, st * P:(st + 1) * P],
                            wv_sb[:, k, :],
                            start=(k == 0), stop=(k == DK - 1))
                    # bias add + scatter into [h, 65]-strided V' slots
                    nc.vector.tensor_add(
                        out=vp_sb[:, st, :, 0:DH],
                        in0=psv[:].rearrange("p (h d) -> p h d", h=HPC),
                        in1=vbias_bc[:].rearrange("p (h d) -> p h d", h=HPC))
                    # ones column times mask == mask itself
                    nc.vector.tensor_copy(
                        out=vp_sb[:, st, :, DH:DH + 1], in_=vmask_sb[:, st, :])
                    for h in range(HPC):
                        nc.vector.tensor_scalar_mul(
                            out=vp_sb[:, st, h, 0:DH], in0=vp_sb[:, st, h, 0:DH],
                            scalar1=vmask_sb[:, st, h:h + 1])

            # ---- attention + inline normalize/output projection ----
            # qb-outer so each q-block's normalize + Y projection can be
            # emitted one head-iteration behind, overlapping the next
            # block's attention and keeping the PE dense (and HAM-warm)
            kt_groups = [list(range(g * G, min(g * G + G, SP))) for g in range((SP + G - 1) // G)]

            sb_pools_cm = [
                tc.tile_pool(name="exps_pool", bufs=3),
                tc.tile_pool(name="st65_pool", bufs=2),
                tc.tile_pool(name="bcast_pool", bufs=2),
                tc.tile_pool(name="y_pool", bufs=2),
            ]
            exps_pool, st65_pool, bcast_pool, y_pool = [c.__enter__() for c in sb_pools_cm]
            with tc.tile_pool(name="stage_psum", bufs=2, space="PSUM") as stage_psum, \
                 tc.tile_pool(name="op_psum", bufs=1, space="PSUM") as op_psum, \
                 tc.tile_pool(name="y_psum", bufs=1, space="PSUM") as y_psum:

                def emit_pv(p):
                    op_p, h_p, kts_p, ex_p = p
                    for i, kt in enumerate(kts_p):
                        nc.tensor.matmul(
                            op_p[:], vp_sb[:, kt, h_p, :], ex_p[:, i, :],
                            start=(kt == 0), stop=(kt == SP - 1),
                            skip_group_check=True)

                def emit_evict(p):
                    op_p, h_p, qb_p = p
                    qs = slice(qb_p * NW, (qb_p + 1) * NW)
                    mt, po = h_p // 2, (h_p % 2) * DH
                    st65 = st65_pool.tile([DH + 1, NW], f32, name="st65")
                    nc.vector.tensor_copy(out=st65[:], in_=op_p[:])
                    nc.sync.dma_start(
                        out=ot_sb[po:po + DH, mt, qs], in_=st65[0:DH, :].bitcast(f32r))
                    nc.sync.dma_start(
                        out=dden[h_p * NB + qb_p, :], in_=st65[DH:DH + 1, :])

                def emit_normalize(qb_p, mt):
                    qs = slice(qb_p * NW, (qb_p + 1) * NW)
                    bc = bcast_pool.tile([P, NW], f32, name="bc")
                    for half in range(2):
                        hh = 2 * mt + half
                        den_row = dden[hh * NB + qb_p:hh * NB + qb_p + 1, :]
                        den_bcast = bass.AP(
                            tensor=den_row.tensor,
                            offset=den_row.offset,
                            ap=[[0, DH]] + list(den_row.ap[1:]),
                        )
                        nc.sync.dma_start(
                            out=bc[half * DH:(half + 1) * DH, :], in_=den_bcast)
                    nc.vector.reciprocal(out=bc[:], in_=bc[:])
                    nc.vector.tensor_mul(
                        out=ot_sb[:, mt, qs],
                        in0=ot_sb[:, mt, qs], in1=bc[:])

                def emit_y_st(st):
                    yt = y_pool.tile([P, D], f32, name="yt")
                    for n2 in range(2):
                        yps = y_psum.tile([P, NW], f32, name="yps")
                        for k2 in range(CW // P):
                            nc.tensor.matmul(
                                yps[:], ot_sb[:, k2, st * P:(st + 1) * P],
                                wo_sb[:, k2, n2 * NW:(n2 + 1) * NW],
                                start=(k2 == 0), stop=(k2 == CW // P - 1))
                        nc.vector.tensor_copy(
                            out=yt[:, n2 * NW:(n2 + 1) * NW], in_=yps[:])
                    nc.sync.dma_start(out=y[st * P:(st + 1) * P, :], in_=yt[:])

                from collections import deque
                pends = deque()     # (op_ps, h, kts, ex, last, qb)
                def drain_one():
                    op_p, h_p, kts_p, ex_p, last, qb_p = pends.popleft()
                    emit_pv((op_p, h_p, kts_p, ex_p))
                    if last:
                        emit_evict((op_p, h_p, qb_p))

                for qb in range(NB):
                    qs = slice(qb * NW, (qb + 1) * NW)
                    for h in range(HPC):
                        mt = h // 2
                        if h == 1 and qb > 0:
                            emit_normalize(qb - 1, 1)
                        if h == 3:
                            emit_normalize(qb, 0)
                        op_ps = op_psum.tile([DH + 1, NW], f32, name="op_ps")
                        for gi, kts in enumerate(kt_groups):
                            ng = len(kts)
                            st_ps = stage_psum.tile([P, G, NW], f32, name="st_ps")
                            for i, kt in enumerate(kts):
                                nc.tensor.matmul(
                                    st_ps[:, i, :],
                                    kz_sb[:, h, kt, :],
                                    qt_sb[:, mt, qs],
                                    start=True, stop=True)
                            if len(pends) >= 1:
                                drain_one()
                            if qb > 0 and h >= 2 and gi in (1, 3):
                                sts = 4 * (qb - 1) + 2 * (h - 2) + (gi - 1) // 2
                                emit_y_st(sts)
                            ex = exps_pool.tile([P, G, NW], f32r, name="ex")
                            nc.scalar.activation(
                                out=ex[:, 0:ng, :], in_=st_ps[:, 0:ng, :],
                                func=Exp, scale=1.0 / np.sqrt(DH))
                            pends.append((op_ps, h, kts, ex, gi == len(kt_groups) - 1, qb))
                while pends:
                    drain_one()

            # tail: last q-block's remaining normalize + output projection,
            # with the attention PSUM banks freed for deeper Y buffering
            with tc.tile_pool(name="y2_psum", bufs=4, space="PSUM") as y2_psum:
                emit_normalize(NB - 1, 1)
                for st in range(4 * (NB - 1), 4 * NB):
                    yt = y_pool.tile([P, D], f32, name="yt")
                    for n2 in range(2):
                        yps = y2_psum.tile([P, NW], f32, name="yps")
                        for k2 in range(CW // P):
                            nc.tensor.matmul(
                                yps[:], ot_sb[:, k2, st * P:(st + 1) * P],
                                wo_sb[:, k2, n2 * NW:(n2 + 1) * NW],
                                start=(k2 == 0), stop=(k2 == CW // P - 1))
                        nc.vector.tensor_copy(
                            out=yt[:, n2 * NW:(n2 + 1) * NW], in_=yps[:])
                    nc.sync.dma_start(out=y[st * P:(st + 1) * P, :], in_=yt[:])

            for c in reversed(sb_pools_cm):
                c.__exit__(None, None, None)
            consts_cm.__exit__(None, None, None)

    nc.compile()
    return nc


def _get_nc():
    if "nc" not in _STATE:
        _STATE["nc"] = _build_nc()
    return _STATE["nc"]


def _make_in_maps(hidden_states, attention_mask, W_q, b_q, W_k, b_k, W_v, b_v, W_o):
    hs = np.asarray(hidden_states, dtype=np.float32)
    mask = np.asarray(attention_mask)
    W_q = np.asarray(W_q, dtype=np.float32)
    W_k = np.asarray(W_k, dtype=np.float32)
    W_v = np.asarray(W_v, dtype=np.float32)
    W_o = np.asarray(W_o, dtype=np.float32)
    b_q = np.asarray(b_q, dtype=np.float32)
    b_k = np.asarray(b_k, dtype=np.float32)
    b_v = np.asarray(b_v, dtype=np.float32)

    in_maps = []
    for c in range(NCORES):
        b, j = c // (NCORES // B), c % (NCORES // B)
        cols = slice(CW * j, CW * (j + 1))
        xt = np.ascontiguousarray(hs[b].T)                                   # [D, S]
        wq = np.ascontiguousarray(W_q[:, cols].reshape(DK, P, CW).transpose(1, 0, 2))
        wk = np.ascontiguousarray(W_k[:, cols].reshape(DK, P, CW).transpose(1, 0, 2))
        wv = np.ascontiguousarray(W_v[:, cols].reshape(DK, P, CW).transpose(1, 0, 2))
        wo = np.ascontiguousarray(W_o[cols, :].reshape(CW // P, P, D).transpose(1, 0, 2))
        bqc = np.ascontiguousarray(b_q[cols].reshape(CW // P, P).T)          # [128, 2]
        bkc = np.ascontiguousarray(b_k[cols].reshape(CW // P, P).T)
        bvc = np.ascontiguousarray(b_v[cols].reshape(1, CW))
        m = mask[b * H + HPC * j: b * H + HPC * (j + 1), 0, :].astype(np.float32)  # [4, S]
        vm = np.ascontiguousarray(m.reshape(HPC, SP, P).transpose(2, 1, 0))  # [128, 16, 4]
        in_maps.append({
            "xt": xt, "wq": wq, "wk": wk, "wv": wv, "wo": wo,
            "bq": bqc, "bk": bkc, "bv": bvc, "vmask": vm,
        })
    return in_maps


def run(inputs, trace=False, **trace_kwargs):
    """Run the SPMD kernel. Returns (full_output, BassKernelResults)."""
    from concourse.bass_utils import run_bass_kernel_spmd

    nc = _get_nc()
    in_maps = _make_in_maps(
        inputs["hidden_states"], inputs["attention_mask"],
        inputs["W_q"], inputs["b_q"], inputs["W_k"], inputs["b_k"],
        inputs["W_v"], inputs["b_v"], inputs["W_o"])
    res = run_bass_kernel_spmd(
        nc, in_maps, list(range(NCORES)), trace=trace, **trace_kwargs)

    b_o = np.asarray(inputs["b_o"], dtype=np.float32)
    out = np.zeros((B, S, D), dtype=np.float32)
    gpb = NCORES // B
    for c in range(NCORES):
        out[c // gpb] += res.results[c]["y"]
    out += b_o[None, None, :]
    return out, res


def kernel(**inputs):
    out, _ = run(inputs, trace=False)
    return out


# revision 16
# speedup vs baseline: 1.1595x; 1.0162x over previous
"""BERT self-attention on 8 Trainium2 NeuronCores.

Sharding: data-parallel over batch (4 cores per batch element) x
tensor-parallel over heads (4 heads per core). Q/K/V projections are
column-sharded, the output projection is row-sharded; each core returns a
partial [S, D] output which the host sums (+ b_o).

Per-core math (batch b, heads hs = 4 heads, cols = 256 feature slice):
  QT = Wq_sl.T @ X_b.T        [256, 2048]   (bias added per-partition)
  KT = Wk_sl.T @ X_b.T        [256, 2048]
  V  = X_b @ Wv_sl            [2048, 256] -> V' = [V_h | 1] per head, mask
                              folded in multiplicatively
  per head h, q-block qb (512 wide):
    S^T tile [128k, 512q] = K_h @ Q_h^T slices  (PSUM)
    expS = exp(S^T / 8)                          (ACT, PSUM->SBUF)
    O'   = V'_h.T @ expS   accumulated over 16 k-tiles -> [65, 512]
           rows 0:64 = unnormalized O_h^T, row 64 = softmax denominator
  normalize: O^T *= 1/den (denominator broadcast across partitions via a
  stride-0 DMA read from a DRAM staging buffer)
  Y_partial = O^T.T @ Wo_sl   [2048, 1024]

All matmuls run as float32r (FP22 reduced precision, full PE rate,
fp32 PSUM accumulation).
"""

import sys

for _p in ("/root/.axon_site/_ro/trn_rl_repo", "/opt/trn_rl_repo"):
    if _p not in sys.path:
        sys.path.append(_p)

import numpy as np
import ml_dtypes

BF16 = ml_dtypes.bfloat16

B, S, D, H, DH = 2, 2048, 1024, 16, 64
P = 128
NCORES = 8
HPC = 4              # heads per core
CW = HPC * DH        # 256: per-core feature slice width
DK = D // P          # 8 k-tiles over the model dim
SP = S // P          # 16 s-tiles
NB = 4               # 512-wide blocks over S
NW = S // NB         # 512
G = 3                # exp kt-group size (PSUM banks per stage tile)

_STATE = {}


def _build_nc():
    import concourse.bacc as bacc
    import concourse.tile as tile
    from concourse import mybir

    f32 = mybir.dt.float32
    f32r = mybir.dt.float32r
    bf16 = mybir.dt.bfloat16
    Exp = mybir.ActivationFunctionType.Exp

    nc = bacc.Bacc(None, target_bir_lowering=False, debug=False)

    with tile.TileContext(nc) as tc:
        with tc.tile_pool(name="dram", bufs=1, space="DRAM") as dram:
            xt = dram.tile([D, S], bf16, kind="ExternalInput", name="xt", uniquify=False)
            wq = dram.tile([P, DK, CW], bf16, kind="ExternalInput", name="wq", uniquify=False)
            wk = dram.tile([P, DK, CW], bf16, kind="ExternalInput", name="wk", uniquify=False)
            wv = dram.tile([P, DK, CW], bf16, kind="ExternalInput", name="wv", uniquify=False)
            wo = dram.tile([P, CW // P, D], f32, kind="ExternalInput", name="wo", uniquify=False)
            bq = dram.tile([P, CW // P], f32, kind="ExternalInput", name="bq", uniquify=False)
            bk = dram.tile([P, CW // P], f32, kind="ExternalInput", name="bk", uniquify=False)
            bv = dram.tile([1, CW], f32, kind="ExternalInput", name="bv", uniquify=False)
            vmask = dram.tile([P, SP, HPC], f32, kind="ExternalInput", name="vmask", uniquify=False)
            y = dram.tile([S, D], f32, kind="ExternalOutput", name="y", uniquify=False)
            dden = dram.tile([HPC * NB, NW], f32, name="dden")

            import concourse.bass as bass

            consts_cm = tc.tile_pool(name="consts", bufs=1)
            consts = consts_cm.__enter__()
            xt_sb = consts.tile([P, DK, S], bf16, name="xt_sb")
            wq_sb = consts.tile([P, DK, CW], bf16, name="wq_sb")
            wk_sb = consts.tile([P, DK, CW], bf16, name="wk_sb")
            wv_sb = consts.tile([P, DK, CW], bf16, name="wv_sb")
            wo_sb = consts.tile([P, CW // P, D], f32r, name="wo_sb")
            bq_sb = consts.tile([P, CW // P], f32, name="bq_sb")
            bk_sb = consts.tile([P, CW // P], f32, name="bk_sb")
            vbias_bc = consts.tile([P, CW], f32, name="vbias_bc")
            vmask_sb = consts.tile([P, SP, HPC], f32, name="vmask_sb")
            qt_sb = consts.tile([P, CW // P, S], bf16, name="qt_sb")
            kz_sb = consts.tile([P, HPC, SP, P], bf16, name="kz_sb")
            zsrc = consts.tile([P, NW], f32, name="zsrc")
            vp_sb = consts.tile([P, SP, HPC, DH + 1], bf16, name="vp_sb")
            ot_sb = consts.tile([P, CW // P, S], f32r, name="ot_sb")

            # ---- input DMAs (small tensors first so the first matmuls
            # aren't queued behind the 8MB of X^T) ----
            nc.sync.dma_start(out=wq_sb[:], in_=wq[:])
            nc.sync.dma_start(out=bq_sb[:], in_=bq[:])
            for k in range(DK):
                nc.sync.dma_start(out=xt_sb[:, k, :], in_=xt[k * P:(k + 1) * P, :])
            nc.sync.dma_start(out=wk_sb[:], in_=wk[:])
            nc.sync.dma_start(out=wv_sb[:], in_=wv[:])
            nc.sync.dma_start(out=wo_sb[:], in_=wo[:].bitcast(f32r))
            nc.sync.dma_start(out=bk_sb[:], in_=bk[:])
            nc.sync.dma_start(out=vmask_sb[:], in_=vmask[:])
            # broadcast b_v row across all 128 partitions (stride-0 DMA read)
            bv_row = bv[0:1, :]
            bv_bcast = bass.AP(
                tensor=bv_row.tensor,
                offset=bv_row.offset,
                ap=[[0, P]] + list(bv_row.ap[1:]),
            )
            nc.sync.dma_start(out=vbias_bc[:], in_=bv_bcast)

            # zero-fill kz (stride-0 free-dim broadcast of a zeroed tile);
            # each head's K^T occupies its natural 64 partition rows, the
            # other 64 rows stay zero so the scores matmul contracts over
            # the full 128 partitions (HAM keeps the PE clock warm only
            # for full-row matmuls)
            nc.vector.memset(zsrc[:], 0.0)
            zview = zsrc[:]
            zbc = bass.AP(
                tensor=zview.tensor, offset=zview.offset,
                ap=[list(zview.ap[0]), [0, HPC * SP * P // NW]] + list(zview.ap[1:]))
            nc.vector.tensor_copy(out=kz_sb[:], in_=zbc)

            # ---- projections ----
            # QT first, k-outer (8 live PSUM accumulators) so matmuls pace
            # with the X^T DMA stream instead of waiting for all of it
            with tc.tile_pool(name="proj_psum", bufs=8, space="PSUM") as proj_psum:
                psqs = {}
                for m in range(CW // P):
                    for nb in range(NB):
                        psqs[(m, nb)] = proj_psum.tile([P, NW], f32, name="psq", tag="pp")
                for k in range(DK):
                    for m in range(CW // P):
                        for nb in range(NB):
                            nc.tensor.matmul(
                                psqs[(m, nb)][:], wq_sb[:, k, m * P:(m + 1) * P],
                                xt_sb[:, k, nb * NW:(nb + 1) * NW],
                                start=(k == 0), stop=(k == DK - 1))
                for m in range(CW // P):
                    for nb in range(NB):
                        nc.vector.tensor_scalar_add(
                            out=qt_sb[:, m, nb * NW:(nb + 1) * NW],
                            in0=psqs[(m, nb)][:], scalar1=bq_sb[:, m:m + 1])

            with tc.tile_pool(name="kproj_psum", bufs=4, space="PSUM") as kproj_psum, \
                 tc.tile_pool(name="vproj_psum", bufs=4, space="PSUM") as vproj_psum:
                for m in range(CW // P):
                    for nb in range(NB):
                        psk = kproj_psum.tile([P, NW], f32, name="psk")
                        for k in range(DK):
                            nc.tensor.matmul(
                                psk[:], wk_sb[:, k, m * P:(m + 1) * P],
                                xt_sb[:, k, nb * NW:(nb + 1) * NW],
                                start=(k == 0), stop=(k == DK - 1))
                        nc.vector.tensor_scalar_add(
                            out=kz_sb[0:DH, 2 * m, nb * 4:(nb + 1) * 4, :],
                            in0=psk[0:DH, :].rearrange("p (a b) -> p a b", a=4),
                            scalar1=bk_sb[0:DH, m:m + 1])
                        nc.vector.tensor_scalar_add(
                            out=kz_sb[DH:P, 2 * m + 1, nb * 4:(nb + 1) * 4, :],
                            in0=psk[DH:P, :].rearrange("p (a b) -> p a b", a=4),
                            scalar1=bk_sb[DH:P, m:m + 1])

                for st in range(SP):            # V s-tiles
                    psv = vproj_psum.tile([P, CW], f32, name="psv")
                    for k in range(DK):
                        nc.tensor.matmul(
                            psv[:], xt_sb[:, k, st * P:(st + 1) * P],
                            wv_sb[:, k, :],
                            start=(k == 0), stop=(k == DK - 1))
                    # bias add + scatter into [h, 65]-strided V' slots
                    nc.vector.tensor_add(
                        out=vp_sb[:, st, :, 0:DH],
                        in0=psv[:].rearrange("p (h d) -> p h d", h=HPC),
                        in1=vbias_bc[:].rearrange("p (h d) -> p h d", h=HPC))
                    # ones column times mask == mask itself
                    nc.vector.tensor_copy(
                        out=vp_sb[:, st, :, DH:DH + 1], in_=vmask_sb[:, st, :])
                    for h in range(HPC):
                        nc.vector.tensor_scalar_mul(
                            out=vp_sb[:, st, h, 0:DH], in0=vp_sb[:, st, h, 0:DH],
                            scalar1=vmask_sb[:, st, h:h + 1])

            # ---- attention + inline normalize/output projection ----
            # qb-outer so each q-block's normalize + Y projection can be
            # emitted one head-iteration behind, overlapping the next
            # block's attention and keeping the PE dense (and HAM-warm)
            kt_groups = [list(range(g * G, min(g * G + G, SP))) for g in range((SP + G - 1) // G)]

            sb_pools_cm = [
                tc.tile_pool(name="exps_pool", bufs=3),
                tc.tile_pool(name="st65_pool", bufs=2),
                tc.tile_pool(name="bcast_pool", bufs=2),
                tc.tile_pool(name="y_pool", bufs=2),
            ]
            exps_pool, st65_pool, bcast_pool, y_pool = [c.__enter__() for c in sb_pools_cm]
            with tc.tile_pool(name="stage_psum", bufs=2, space="PSUM") as stage_psum, \
                 tc.tile_pool(name="op_psum", bufs=1, space="PSUM") as op_psum, \
                 tc.tile_pool(name="y_psum", bufs=1, space="PSUM") as y_psum:

                def emit_pv(p):
                    op_p, h_p, kts_p, ex_p = p
                    for i, kt in enumerate(kts_p):
                        nc.tensor.matmul(
                            op_p[:], vp_sb[:, kt, h_p, :], ex_p[:, i, :],
                            start=(kt == 0), stop=(kt == SP - 1),
                            skip_group_check=True)

                def emit_evict(p):
                    op_p, h_p, qb_p = p
                    qs = slice(qb_p * NW, (qb_p + 1) * NW)
                    mt, po = h_p // 2, (h_p % 2) * DH
                    st65 = st65_pool.tile([DH + 1, NW], f32, name="st65")
                    nc.vector.tensor_copy(out=st65[:], in_=op_p[:])
                    nc.sync.dma_start(
                        out=ot_sb[po:po + DH, mt, qs], in_=st65[0:DH, :].bitcast(f32r))
                    nc.sync.dma_start(
                        out=dden[h_p * NB + qb_p, :], in_=st65[DH:DH + 1, :])

                def emit_normalize(qb_p, mt):
                    qs = slice(qb_p * NW, (qb_p + 1) * NW)
                    bc = bcast_pool.tile([P, NW], f32, name="bc")
                    for half in range(2):
                        hh = 2 * mt + half
                        den_row = dden[hh * NB + qb_p:hh * NB + qb_p + 1, :]
                        den_bcast = bass.AP(
                            tensor=den_row.tensor,
                            offset=den_row.offset,
                            ap=[[0, DH]] + list(den_row.ap[1:]),
                        )
                        nc.sync.dma_start(
                            out=bc[half * DH:(half + 1) * DH, :], in_=den_bcast)
                    nc.vector.reciprocal(out=bc[:], in_=bc[:])
                    nc.vector.tensor_mul(
                        out=ot_sb[:, mt, qs],
                        in0=ot_sb[:, mt, qs], in1=bc[:])

                def emit_y_st(st):
                    yt = y_pool.tile([P, D], f32, name="yt")
                    for n2 in range(2):
                        yps = y_psum.tile([P, NW], f32, name="yps")
                        for k2 in range(CW // P):
                            nc.tensor.matmul(
                                yps[:], ot_sb[:, k2, st * P:(st + 1) * P],
                                wo_sb[:, k2, n2 * NW:(n2 + 1) * NW],
                                start=(k2 == 0), stop=(k2 == CW // P - 1))
                        nc.vector.tensor_copy(
                            out=yt[:, n2 * NW:(n2 + 1) * NW], in_=yps[:])
                    nc.sync.dma_start(out=y[st * P:(st + 1) * P, :], in_=yt[:])

                from collections import deque
                pends = deque()     # (op_ps, h, kts, ex, last, qb)
                def drain_one():
                    op_p, h_p, kts_p, ex_p, last, qb_p = pends.popleft()
                    emit_pv((op_p, h_p, kts_p, ex_p))
                    if last:
                        emit_evict((op_p, h_p, qb_p))

                for qb in range(NB):
                    qs = slice(qb * NW, (qb + 1) * NW)
                    for h in range(HPC):
                        mt = h // 2
                        if h == 1 and qb > 0:
                            emit_normalize(qb - 1, 1)
                        if h == 3:
                            emit_normalize(qb, 0)
                        op_ps = op_psum.tile([DH + 1, NW], f32, name="op_ps")
                        for gi, kts in enumerate(kt_groups):
                            ng = len(kts)
                            st_ps = stage_psum.tile([P, G, NW], f32, name="st_ps")
                            for i, kt in enumerate(kts):
                                nc.tensor.matmul(
                                    st_ps[:, i, :],
                                    kz_sb[:, h, kt, :],
                                    qt_sb[:, mt, qs],
                                    start=True, stop=True)
                            if len(pends) >= 1:
                                drain_one()
                            if qb > 0 and h >= 2 and gi in (1, 3):
                                sts = 4 * (qb - 1) + 2 * (h - 2) + (gi - 1) // 2
                                emit_y_st(sts)
                            ex = exps_pool.tile([P, G, NW], bf16, name="ex")
                            nc.scalar.activation(
                                out=ex[:, 0:ng, :], in_=st_ps[:, 0:ng, :],
                                func=Exp, scale=1.0 / np.sqrt(DH))
                            pends.append((op_ps, h, kts, ex, gi == len(kt_groups) - 1, qb))
                while pends:
                    drain_one()

            # tail: last q-block's remaining normalize + output projection,
            # with the attention PSUM banks freed for deeper Y buffering
            with tc.tile_pool(name="y2_psum", bufs=4, space="PSUM") as y2_psum:
                emit_normalize(NB - 1, 1)
                for st in range(4 * (NB - 1), 4 * NB):
                    yt = y_pool.tile([P, D], f32, name="yt")
                    for n2 in range(2):
                        yps = y2_psum.tile([P, NW], f32, name="yps")
                        for k2 in range(CW // P):
                            nc.tensor.matmul(
                                yps[:], ot_sb[:, k2, st * P:(st + 1) * P],
                                wo_sb[:, k2, n2 * NW:(n2 + 1) * NW],
                                start=(k2 == 0), stop=(k2 == CW // P - 1))
                        nc.vector.tensor_copy(
                            out=yt[:, n2 * NW:(n2 + 1) * NW], in_=yps[:])
                    nc.sync.dma_start(out=y[st * P:(st + 1) * P, :], in_=yt[:])

            for c in reversed(sb_pools_cm):
                c.__exit__(None, None, None)
            consts_cm.__exit__(None, None, None)

    nc.compile()
    return nc


def _get_nc():
    if "nc" not in _STATE:
        _STATE["nc"] = _build_nc()
    return _STATE["nc"]


def _make_in_maps(hidden_states, attention_mask, W_q, b_q, W_k, b_k, W_v, b_v, W_o):
    hs = np.asarray(hidden_states, dtype=np.float32)
    mask = np.asarray(attention_mask)
    W_q = np.asarray(W_q, dtype=np.float32)
    W_k = np.asarray(W_k, dtype=np.float32)
    W_v = np.asarray(W_v, dtype=np.float32)
    W_o = np.asarray(W_o, dtype=np.float32)
    b_q = np.asarray(b_q, dtype=np.float32)
    b_k = np.asarray(b_k, dtype=np.float32)
    b_v = np.asarray(b_v, dtype=np.float32)

    in_maps = []
    for c in range(NCORES):
        b, j = c // (NCORES // B), c % (NCORES // B)
        cols = slice(CW * j, CW * (j + 1))
        xt = np.ascontiguousarray(hs[b].T.astype(BF16))                      # [D, S]
        wq = np.ascontiguousarray(W_q[:, cols].reshape(DK, P, CW).transpose(1, 0, 2).astype(BF16))
        wk = np.ascontiguousarray(W_k[:, cols].reshape(DK, P, CW).transpose(1, 0, 2).astype(BF16))
        wv = np.ascontiguousarray(W_v[:, cols].reshape(DK, P, CW).transpose(1, 0, 2).astype(BF16))
        wo = np.ascontiguousarray(W_o[cols, :].reshape(CW // P, P, D).transpose(1, 0, 2))
        bqc = np.ascontiguousarray(b_q[cols].reshape(CW // P, P).T)          # [128, 2]
        bkc = np.ascontiguousarray(b_k[cols].reshape(CW // P, P).T)
        bvc = np.ascontiguousarray(b_v[cols].reshape(1, CW))
        m = mask[b * H + HPC * j: b * H + HPC * (j + 1), 0, :].astype(np.float32)  # [4, S]
        vm = np.ascontiguousarray(m.reshape(HPC, SP, P).transpose(2, 1, 0))  # [128, 16, 4]
        in_maps.append({
            "xt": xt, "wq": wq, "wk": wk, "wv": wv, "wo": wo,
            "bq": bqc, "bk": bkc, "bv": bvc, "vmask": vm,
        })
    return in_maps


def run(inputs, trace=False, **trace_kwargs):
    """Run the SPMD kernel. Returns (full_output, BassKernelResults)."""
    from concourse.bass_utils import run_bass_kernel_spmd

    nc = _get_nc()
    in_maps = _make_in_maps(
        inputs["hidden_states"], inputs["attention_mask"],
        inputs["W_q"], inputs["b_q"], inputs["W_k"], inputs["b_k"],
        inputs["W_v"], inputs["b_v"], inputs["W_o"])
    res = run_bass_kernel_spmd(
        nc, in_maps, list(range(NCORES)), trace=trace, **trace_kwargs)

    b_o = np.asarray(inputs["b_o"], dtype=np.float32)
    out = np.zeros((B, S, D), dtype=np.float32)
    gpb = NCORES // B
    for c in range(NCORES):
        out[c // gpb] += res.results[c]["y"]
    out += b_o[None, None, :]
    return out, res


def kernel(**inputs):
    out, _ = run(inputs, trace=False)
    return out


# revision 18
# speedup vs baseline: 1.1667x; 1.0063x over previous
"""BERT self-attention on 8 Trainium2 NeuronCores.

Sharding: data-parallel over batch (4 cores per batch element) x
tensor-parallel over heads (4 heads per core). Q/K/V projections are
column-sharded, the output projection is row-sharded; each core returns a
partial [S, D] output which the host sums (+ b_o).

Per-core math (batch b, heads hs = 4 heads, cols = 256 feature slice):
  QT = Wq_sl.T @ X_b.T        [256, 2048]   (bias added per-partition)
  KT = Wk_sl.T @ X_b.T        [256, 2048]
  V  = X_b @ Wv_sl            [2048, 256] -> V' = [V_h | 1] per head, mask
                              folded in multiplicatively
  per head h, q-block qb (512 wide):
    S^T tile [128k, 512q] = K_h @ Q_h^T slices  (PSUM)
    expS = exp(S^T / 8)                          (ACT, PSUM->SBUF)
    O'   = V'_h.T @ expS   accumulated over 16 k-tiles -> [65, 512]
           rows 0:64 = unnormalized O_h^T, row 64 = softmax denominator
  normalize: O^T *= 1/den (denominator broadcast across partitions via a
  stride-0 DMA read from a DRAM staging buffer)
  Y_partial = O^T.T @ Wo_sl   [2048, 1024]

Matmuls accumulate in fp32 PSUM; the QK/QT path and V'/expS run in bf16
(full PE rate + fast weight load), the output projection in float32r
(FP22). K tiles are zero-padded to the full 128 contraction rows because
the PE clock gate (HAM) only unthrottles for full-row matmuls.
"""

import sys

for _p in ("/root/.axon_site/_ro/trn_rl_repo", "/opt/trn_rl_repo"):
    if _p not in sys.path:
        sys.path.append(_p)

import numpy as np
import ml_dtypes

BF16 = ml_dtypes.bfloat16

B, S, D, H, DH = 2, 2048, 1024, 16, 64
P = 128
NCORES = 8
HPC = 4              # heads per core
CW = HPC * DH        # 256: per-core feature slice width
DK = D // P          # 8 k-tiles over the model dim
SP = S // P          # 16 s-tiles
NB = 4               # 512-wide blocks over S
NW = S // NB         # 512
G = 3                # exp kt-group size (PSUM banks per stage tile)

_STATE = {}


def _build_nc():
    import concourse.bacc as bacc
    import concourse.tile as tile
    from concourse import mybir

    f32 = mybir.dt.float32
    f32r = mybir.dt.float32r
    bf16 = mybir.dt.bfloat16
    Exp = mybir.ActivationFunctionType.Exp

    nc = bacc.Bacc(None, target_bir_lowering=False, debug=False)

    with tile.TileContext(nc) as tc:
        with tc.tile_pool(name="dram", bufs=1, space="DRAM") as dram:
            xt = dram.tile([D, S], bf16, kind="ExternalInput", name="xt", uniquify=False)
            wq = dram.tile([P, DK, CW], bf16, kind="ExternalInput", name="wq", uniquify=False)
            wk = dram.tile([P, DK, CW], bf16, kind="ExternalInput", name="wk", uniquify=False)
            wv = dram.tile([P, DK, CW], bf16, kind="ExternalInput", name="wv", uniquify=False)
            wo = dram.tile([P, CW // P, D], f32, kind="ExternalInput", name="wo", uniquify=False)
            bq = dram.tile([P, CW // P], f32, kind="ExternalInput", name="bq", uniquify=False)
            bk = dram.tile([P, CW // P], f32, kind="ExternalInput", name="bk", uniquify=False)
            bv = dram.tile([1, CW], f32, kind="ExternalInput", name="bv", uniquify=False)
            vmask = dram.tile([P, SP, HPC], f32, kind="ExternalInput", name="vmask", uniquify=False)
            y = dram.tile([S, D], f32, kind="ExternalOutput", name="y", uniquify=False)
            dden = dram.tile([HPC * NB, NW], f32, name="dden")

            import concourse.bass as bass

            consts_cm = tc.tile_pool(name="consts", bufs=1)
            consts = consts_cm.__enter__()
            xt_sb = consts.tile([P, DK, S], bf16, name="xt_sb")
            wq_sb = consts.tile([P, DK, CW], bf16, name="wq_sb")
            wk_sb = consts.tile([P, DK, CW], bf16, name="wk_sb")
            wv_sb = consts.tile([P, DK, CW], bf16, name="wv_sb")
            wo_sb = consts.tile([P, CW // P, D], f32r, name="wo_sb")
            bq_sb = consts.tile([P, CW // P], f32, name="bq_sb")
            bk_sb = consts.tile([P, CW // P], f32, name="bk_sb")
            vbias_bc = consts.tile([P, CW], f32, name="vbias_bc")
            vmask_sb = consts.tile([P, SP, HPC], f32, name="vmask_sb")
            qt_sb = consts.tile([P, CW // P, S], bf16, name="qt_sb")
            kz_sb = consts.tile([P, HPC, SP, P], bf16, name="kz_sb")
            zsrc = consts.tile([P, NW], f32, name="zsrc")
            vp_sb = consts.tile([P, SP, HPC, DH + 1], bf16, name="vp_sb")
            ot_sb = consts.tile([P, CW // P, S], f32r, name="ot_sb")

            # ---- input DMAs (small tensors first so the first matmuls
            # aren't queued behind the 8MB of X^T) ----
            nc.sync.dma_start(out=wq_sb[:], in_=wq[:])
            nc.sync.dma_start(out=bq_sb[:], in_=bq[:])
            for k in range(DK):
                for hlf in range(2):
                    hs_ = slice(hlf * (S // 2), (hlf + 1) * (S // 2))
                    nc.sync.dma_start(out=xt_sb[:, k, hs_],
                                      in_=xt[k * P:(k + 1) * P, hs_])
            nc.sync.dma_start(out=wk_sb[:], in_=wk[:])
            nc.sync.dma_start(out=wv_sb[:], in_=wv[:])
            nc.sync.dma_start(out=wo_sb[:], in_=wo[:].bitcast(f32r))
            nc.sync.dma_start(out=bk_sb[:], in_=bk[:])
            nc.sync.dma_start(out=vmask_sb[:], in_=vmask[:])
            # broadcast b_v row across all 128 partitions (stride-0 DMA read)
            bv_row = bv[0:1, :]
            bv_bcast = bass.AP(
                tensor=bv_row.tensor,
                offset=bv_row.offset,
                ap=[[0, P]] + list(bv_row.ap[1:]),
            )
            nc.sync.dma_start(out=vbias_bc[:], in_=bv_bcast)

            # zero-fill kz (stride-0 free-dim broadcast of a zeroed tile);
            # each head's K^T occupies its natural 64 partition rows, the
            # other 64 rows stay zero so the scores matmul contracts over
            # the full 128 partitions (HAM keeps the PE clock warm only
            # for full-row matmuls)
            nc.vector.memset(zsrc[:], 0.0)
            zview = zsrc[:]
            zbc = bass.AP(
                tensor=zview.tensor, offset=zview.offset,
                ap=[list(zview.ap[0]), [0, HPC * SP * P // NW]] + list(zview.ap[1:]))
            nc.vector.tensor_copy(out=kz_sb[:], in_=zbc)

            # ---- projections ----
            # QT first, k-outer (8 live PSUM accumulators) so matmuls pace
            # with the X^T DMA stream instead of waiting for all of it
            with tc.tile_pool(name="proj_psum", bufs=8, space="PSUM") as proj_psum:
                psqs = {}
                for m in range(CW // P):
                    for nb in range(NB):
                        psqs[(m, nb)] = proj_psum.tile([P, NW], f32, name="psq", tag="pp")
                for k in range(DK):
                    for m in range(CW // P):
                        for nb in range(NB):
                            nc.tensor.matmul(
                                psqs[(m, nb)][:], wq_sb[:, k, m * P:(m + 1) * P],
                                xt_sb[:, k, nb * NW:(nb + 1) * NW],
                                start=(k == 0), stop=(k == DK - 1))
                for m in range(CW // P):
                    for nb in range(NB):
                        nc.vector.tensor_scalar_add(
                            out=qt_sb[:, m, nb * NW:(nb + 1) * NW],
                            in0=psqs[(m, nb)][:], scalar1=bq_sb[:, m:m + 1])

            with tc.tile_pool(name="kproj_psum", bufs=4, space="PSUM") as kproj_psum, \
                 tc.tile_pool(name="vproj_psum", bufs=4, space="PSUM") as vproj_psum:
                for m in range(CW // P):
                    for nb in range(NB):
                        psk = kproj_psum.tile([P, NW], f32, name="psk")
                        for k in range(DK):
                            nc.tensor.matmul(
                                psk[:], wk_sb[:, k, m * P:(m + 1) * P],
                                xt_sb[:, k, nb * NW:(nb + 1) * NW],
                                start=(k == 0), stop=(k == DK - 1))
                        nc.vector.tensor_scalar_add(
                            out=kz_sb[0:DH, 2 * m, nb * 4:(nb + 1) * 4, :],
                            in0=psk[0:DH, :].rearrange("p (a b) -> p a b", a=4),
                            scalar1=bk_sb[0:DH, m:m + 1])
                        nc.vector.tensor_scalar_add(
                            out=kz_sb[DH:P, 2 * m + 1, nb * 4:(nb + 1) * 4, :],
                            in0=psk[DH:P, :].rearrange("p (a b) -> p a b", a=4),
                            scalar1=bk_sb[DH:P, m:m + 1])

                for st in range(SP):            # V s-tiles
                    psv = vproj_psum.tile([P, CW], f32, name="psv")
                    for k in range(DK):
                        nc.tensor.matmul(
                            psv[:], xt_sb[:, k, st * P:(st + 1) * P],
                            wv_sb[:, k, :],
                            start=(k == 0), stop=(k == DK - 1))
                    # bias add + scatter into [h, 65]-strided V' slots
                    nc.vector.tensor_add(
                        out=vp_sb[:, st, :, 0:DH],
                        in0=psv[:].rearrange("p (h d) -> p h d", h=HPC),
                        in1=vbias_bc[:].rearrange("p (h d) -> p h d", h=HPC))
                    # ones column times mask == mask itself
                    nc.vector.tensor_copy(
                        out=vp_sb[:, st, :, DH:DH + 1], in_=vmask_sb[:, st, :])
                    for h in range(HPC):
                        nc.vector.tensor_scalar_mul(
                            out=vp_sb[:, st, h, 0:DH], in0=vp_sb[:, st, h, 0:DH],
                            scalar1=vmask_sb[:, st, h:h + 1])

            # ---- attention + inline normalize/output projection ----
            # qb-outer so each q-block's normalize + Y projection can be
            # emitted one head-iteration behind, overlapping the next
            # block's attention and keeping the PE dense (and HAM-warm)
            kt_groups = [list(range(g * G, min(g * G + G, SP))) for g in range((SP + G - 1) // G)]

            sb_pools_cm = [
                tc.tile_pool(name="exps_pool", bufs=4),
                tc.tile_pool(name="st65_pool", bufs=3),
                tc.tile_pool(name="bcast_pool", bufs=2),
                tc.tile_pool(name="y_pool", bufs=2),
            ]
            exps_pool, st65_pool, bcast_pool, y_pool = [c.__enter__() for c in sb_pools_cm]
            with tc.tile_pool(name="stage_psum", bufs=2, space="PSUM") as stage_psum, \
                 tc.tile_pool(name="op_psum", bufs=1, space="PSUM") as op_psum, \
                 tc.tile_pool(name="y_psum", bufs=1, space="PSUM") as y_psum:

                def emit_pv(p):
                    op_p, h_p, kts_p, ex_p = p
                    for i, kt in enumerate(kts_p):
                        nc.tensor.matmul(
                            op_p[:], vp_sb[:, kt, h_p, :], ex_p[:, i, :],
                            start=(kt == 0), stop=(kt == SP - 1),
                            skip_group_check=True)

                def emit_evict(p):
                    op_p, h_p, qb_p = p
                    qs = slice(qb_p * NW, (qb_p + 1) * NW)
                    mt, po = h_p // 2, (h_p % 2) * DH
                    st65 = st65_pool.tile([DH + 1, NW], f32, name="st65")
                    nc.vector.tensor_copy(out=st65[:], in_=op_p[:])
                    nc.sync.dma_start(
                        out=ot_sb[po:po + DH, mt, qs], in_=st65[0:DH, :].bitcast(f32r))
                    nc.sync.dma_start(
                        out=dden[h_p * NB + qb_p, :], in_=st65[DH:DH + 1, :])

                def emit_normalize(qb_p, mt):
                    qs = slice(qb_p * NW, (qb_p + 1) * NW)
                    bc = bcast_pool.tile([P, NW], f32, name="bc")
                    for half in range(2):
                        hh = 2 * mt + half
                        den_row = dden[hh * NB + qb_p:hh * NB + qb_p + 1, :]
                        den_bcast = bass.AP(
                            tensor=den_row.tensor,
                            offset=den_row.offset,
                            ap=[[0, DH]] + list(den_row.ap[1:]),
                        )
                        nc.sync.dma_start(
                            out=bc[half * DH:(half + 1) * DH, :], in_=den_bcast)
                    nc.vector.reciprocal(out=bc[:], in_=bc[:])
                    nc.vector.tensor_mul(
                        out=ot_sb[:, mt, qs],
                        in0=ot_sb[:, mt, qs], in1=bc[:])

                def emit_y_st(st):
                    yt = y_pool.tile([P, D], f32, name="yt")
                    for n2 in range(2):
                        yps = y_psum.tile([P, NW], f32, name="yps")
                        for k2 in range(CW // P):
                            nc.tensor.matmul(
                                yps[:], ot_sb[:, k2, st * P:(st + 1) * P],
                                wo_sb[:, k2, n2 * NW:(n2 + 1) * NW],
                                start=(k2 == 0), stop=(k2 == CW // P - 1))
                        nc.vector.tensor_copy(
                            out=yt[:, n2 * NW:(n2 + 1) * NW], in_=yps[:])
                    nc.sync.dma_start(out=y[st * P:(st + 1) * P, :], in_=yt[:])

                from collections import deque
                pends = deque()     # (op_ps, h, kts, ex, last, qb)
                def drain_one():
                    op_p, h_p, kts_p, ex_p, last, qb_p = pends.popleft()
                    emit_pv((op_p, h_p, kts_p, ex_p))
                    if last:
                        emit_evict((op_p, h_p, qb_p))

                for qb in range(NB):
                    qs = slice(qb * NW, (qb + 1) * NW)
                    for h in range(HPC):
                        mt = h // 2
                        if h == 1 and qb > 0:
                            emit_normalize(qb - 1, 1)
                        if h == 3:
                            emit_normalize(qb, 0)
                        op_ps = op_psum.tile([DH + 1, NW], f32, name="op_ps")
                        for gi, kts in enumerate(kt_groups):
                            ng = len(kts)
                            st_ps = stage_psum.tile([P, G, NW], f32, name="st_ps")
                            for i, kt in enumerate(kts):
                                nc.tensor.matmul(
                                    st_ps[:, i, :],
                                    kz_sb[:, h, kt, :],
                                    qt_sb[:, mt, qs],
                                    start=True, stop=True)
                            if len(pends) >= 1:
                                drain_one()
                            if qb > 0 and h >= 2 and gi in (1, 3):
                                sts = 4 * (qb - 1) + 2 * (h - 2) + (gi - 1) // 2
                                emit_y_st(sts)
                            ex = exps_pool.tile([P, G, NW], bf16, name="ex")
                            nc.scalar.activation(
                                out=ex[:, 0:ng, :], in_=st_ps[:, 0:ng, :],
                                func=Exp, scale=1.0 / np.sqrt(DH))
                            pends.append((op_ps, h, kts, ex, gi == len(kt_groups) - 1, qb))
                while pends:
                    drain_one()

            # tail: last q-block's remaining normalize + output projection,
            # with the attention PSUM banks freed for deeper Y buffering
            with tc.tile_pool(name="y2_psum", bufs=4, space="PSUM") as y2_psum:
                emit_normalize(NB - 1, 1)
                for st in range(4 * (NB - 1), 4 * NB):
                    yt = y_pool.tile([P, D], f32, name="yt")
                    for n2 in range(2):
                        yps = y2_psum.tile([P, NW], f32, name="yps")
                        for k2 in range(CW // P):
                            nc.tensor.matmul(
                                yps[:], ot_sb[:, k2, st * P:(st + 1) * P],
                                wo_sb[:, k2, n2 * NW:(n2 + 1) * NW],
                                start=(k2 == 0), stop=(k2 == CW // P - 1))
                        nc.vector.tensor_copy(
                            out=yt[:, n2 * NW:(n2 + 1) * NW], in_=yps[:])
                    nc.sync.dma_start(out=y[st * P:(st + 1) * P, :], in_=yt[:])

            for c in reversed(sb_pools_cm):
                c.__exit__(None, None, None)
            consts_cm.__exit__(None, None, None)

    nc.compile()
    return nc


def _get_nc():
    if "nc" not in _STATE:
        _STATE["nc"] = _build_nc()
    return _STATE["nc"]


def _make_in_maps(hidden_states, attention_mask, W_q, b_q, W_k, b_k, W_v, b_v, W_o):
    hs = np.asarray(hidden_states, dtype=np.float32)
    mask = np.asarray(attention_mask)
    W_q = np.asarray(W_q, dtype=np.float32)
    W_k = np.asarray(W_k, dtype=np.float32)
    W_v = np.asarray(W_v, dtype=np.float32)
    W_o = np.asarray(W_o, dtype=np.float32)
    b_q = np.asarray(b_q, dtype=np.float32)
    b_k = np.asarray(b_k, dtype=np.float32)
    b_v = np.asarray(b_v, dtype=np.float32)

    in_maps = []
    for c in range(NCORES):
        b, j = c // (NCORES // B), c % (NCORES // B)
        cols = slice(CW * j, CW * (j + 1))
        xt = np.ascontiguousarray(hs[b].T.astype(BF16))                      # [D, S]
        wq = np.ascontiguousarray(W_q[:, cols].reshape(DK, P, CW).transpose(1, 0, 2).astype(BF16))
        wk = np.ascontiguousarray(W_k[:, cols].reshape(DK, P, CW).transpose(1, 0, 2).astype(BF16))
        wv = np.ascontiguousarray(W_v[:, cols].reshape(DK, P, CW).transpose(1, 0, 2).astype(BF16))
        wo = np.ascontiguousarray(W_o[cols, :].reshape(CW // P, P, D).transpose(1, 0, 2))
        bqc = np.ascontiguousarray(b_q[cols].reshape(CW // P, P).T)          # [128, 2]
        bkc = np.ascontiguousarray(b_k[cols].reshape(CW // P, P).T)
        bvc = np.ascontiguousarray(b_v[cols].reshape(1, CW))
        m = mask[b * H + HPC * j: b * H + HPC * (j + 1), 0, :].astype(np.float32)  # [4, S]
        vm = np.ascontiguousarray(m.reshape(HPC, SP, P).transpose(2, 1, 0))  # [128, 16, 4]
        in_maps.append({
            "xt": xt, "wq": wq, "wk": wk, "wv": wv, "wo": wo,
            "bq": bqc, "bk": bkc, "bv": bvc, "vmask": vm,
        })
    return in_maps


def run(inputs, trace=False, **trace_kwargs):
    """Run the SPMD kernel. Returns (full_output, BassKernelResults)."""
    from concourse.bass_utils import run_bass_kernel_spmd

    nc = _get_nc()
    in_maps = _make_in_maps(
        inputs["hidden_states"], inputs["attention_mask"],
        inputs["W_q"], inputs["b_q"], inputs["W_k"], inputs["b_k"],
        inputs["W_v"], inputs["b_v"], inputs["W_o"])
    res = run_bass_kernel_spmd(
        nc, in_maps, list(range(NCORES)), trace=trace, **trace_kwargs)

    b_o = np.asarray(inputs["b_o"], dtype=np.float32)
    out = np.zeros((B, S, D), dtype=np.float32)
    gpb = NCORES // B
    for c in range(NCORES):
        out[c // gpb] += res.results[c]["y"]
    out += b_o[None, None, :]
    return out, res


def kernel(**inputs):
    out, _ = run(inputs, trace=False)
    return out
